# revision 1
# baseline (speedup 1.0000x reference)
"""Trainium2 Bass kernel for nn_Attention_11158325035119.

Reference computation (B=2, N=2048, DIM=1024, H=16, DH=64):
  LayerNorm(x) -> Q,K,V projections -> softmax(Q K^T) V (raw logits, no
  1/sqrt(d) scale) -> output projection.

Sharding over 8 NeuronCores: data-parallel on batch (2 groups of 4 cores),
tensor-parallel on heads within each group (4 heads/core, Wq/Wkv
column-sharded).  Instead of the classic Wout-row-shard + AllReduce (8MB
AllReduce per group, ~100us at the very end), each core's normalized
attention output is redistributed with a per-head AllToAll (overlapped with
the remaining heads' compute) so every core ends up with all heads for a
quarter of its batch's rows and computes a disjoint out-row-slice.  The host
then just concatenates the 8 slices.

The runtime only supports AllToAll on >4-core mesh groups, so the A2A runs
over all 8 cores: each core duplicates its 4 lane-shards into both groups'
slots, and the output projection contracts over a doubled inner dimension
(2048) against a per-core Wout whose other-group row-blocks are zero
(host-prepared).  That keeps the program SPMD (no core-id branching).

Per-core pipeline:
  1. LayerNorm (bn_stats/bn_aggr, rows-on-partitions)
  2. PE-transpose -> xnT [feat, seq], gamma/beta affine fused into the
     PSUM->SBUF copy as per-partition scalars (written as fp32r)
  3. Q^T, K^T = Wq/k^T @ xnT, V = xnT^T @ Wv   (fp32r matmuls, full PE rate)
  4. per head: S^T[k,q] = K Q^T (fp32r); exp on ScalarE (PSUM fp32 ->
     SBUF bf16; no max-subtraction: |logits| < ~50 so exp stays in range);
     O^T_ext = [V|1]^T @ expS (bf16, M=65: row 64 accumulates the softmax
     denominators inside the same matmul)
  5. O^T *= 1/denom (bf16)
  6. AllToAll (per head, 8 cores) : heads <-> q-row-slices
  7. out_slice = O_full^T.T @ Wout2 (bf16) -> fp32 [512, 1024]

gamma/beta are applied generically (they are ones/zeros in this problem's
setup_inputs, but the kernel does not rely on that).
"""

import numpy as np

import concourse.bass as bass
import concourse.tile as tile
from concourse import mybir
from concourse.masks import make_identity

F32 = mybir.dt.float32
F32R = mybir.dt.float32r
BF16 = mybir.dt.bfloat16

EPS = 1e-5

B, N, DIM = 2, 2048, 1024
H, DH = 16, 64
N_CORES = 8
LANES = 4            # cores per batch group (head-parallel)
HL = H // LANES      # local heads per core


# ---------------------------------------------------------------------------
# Environment workarounds
# ---------------------------------------------------------------------------

def _install_drain_split():
    """walrus in this image rejects InstDrain with >1 sem wait ("Too many
    sync wait commands").  Replace the TileContext tail drain with a chain
    of drains, each waiting on a single proc's semaphore."""
    import re
    import bass_rust

    def _split_drain_and_barrier(self, tick_clock, wait_clock):
        nc = self.nc
        gc = tick_clock.global_clock
        ticks = [int(v) for v in re.findall(r"\d+", repr(gc))]
        for proc, t in [(i, t) for i, t in enumerate(ticks) if t > 0]:
            pc = bass_rust.VectorClock()
            pc.require_at_least(proc, t)
            d = nc.sync.drain()
            wait_clock.add_sem_waits(d.ins, bass_rust.ScopedClock({None: pc}))
        nc.all_engine_barrier()
        assert self.sems is not None
        popped = nc._tile_sem_poison_stack.pop()
        assert popped is self._sem_poison
        nc.clear_and_free_semaphores(list(self.sems.allocated().values()))
        nc.all_engine_barrier()

    tile.TileContext._drain_and_barrier = _split_drain_and_barrier


def _install_profile_shim():
    """Provide antenv.axon_hooks (NTFF profiling via libaxon_pjrt.so) and a
    no-op upload_artifacts (no artifact bucket in this container)."""
    import sys
    import types
    import contextlib
    import ctypes
    import os
    import concourse.bass_utils as bu

    if "antenv.axon_hooks" not in sys.modules:
        hook = None
        so_path = "/opt/axon/libaxon_pjrt.so"
        if os.path.exists(so_path):
            lib = ctypes.CDLL(so_path)
            if hasattr(lib, "axon_start_nrt_profile"):
                lib.axon_start_nrt_profile.argtypes = [
                    ctypes.POINTER(ctypes.c_int64), ctypes.c_size_t]
                lib.axon_start_nrt_profile.restype = ctypes.c_int64
                lib.axon_stop_nrt_profile.argtypes = [ctypes.c_char_p]
                lib.axon_stop_nrt_profile.restype = ctypes.c_int64

                @contextlib.contextmanager
                def _hook(output_dir, device_ids):
                    import jax
                    jax.devices()
                    if device_ids:
                        ids = (ctypes.c_int64 * len(device_ids))(*device_ids)
                        rc = lib.axon_start_nrt_profile(ids, len(device_ids))
                    else:
                        rc = lib.axon_start_nrt_profile(None, 0)
                    if rc != 0:
                        raise RuntimeError(f"axon_start_nrt_profile rc={rc}")
                    try:
                        yield
                    finally:
                        lib.axon_stop_nrt_profile(str(output_dir).encode())
                hook = _hook
        mod = types.ModuleType("antenv.axon_hooks")
        mod.get_axon_ntff_profile_hook = lambda: hook
        mod.set_axon_ntff_profile_hook = lambda h: None
        sys.modules["antenv.axon_hooks"] = mod

    bu.upload_artifacts = lambda tmpdir: f"file://{tmpdir}"


_NOPW = [0]


def split_multi_waits(nc):
    """walrus in this image rejects any engine instruction carrying more
    than one semaphore wait ("Too many sync wait commands").  Hoist extra
    waits onto InstNoOps inserted immediately before the instruction on the
    same engine — semantically identical (the waits are a conjunction and
    execute in stream order)."""
    for f in nc.m.functions:
        for blk in f.blocks:
            il = blk.instructions
            i = 0
            while i < len(il):
                inst = il[i]
                si = inst.sync_info
                if si is not None and si.on_wait is not None \
                        and len(si.on_wait) > 1:
                    waits = list(si.on_wait)
                    inst.sync_info = mybir.SyncInfo(
                        on_wait=[waits[-1]],
                        on_update=list(si.on_update or []))
                    for w in waits[:-1]:
                        _NOPW[0] += 1
                        nop = mybir.InstNoOp(name=f"nopw-{_NOPW[0]}")
                        nop.engine = inst.engine
                        nop.sync_info = mybir.SyncInfo(on_wait=[w],
                                                       on_update=[])
                        il.insert(i, nop)
                        i += 1
                i += 1
    return nc


def _install_neff_cache():
    """Disk-cache walrus NEFF compiles by bir_json content hash (a fresh
    process otherwise pays the full 10-25 min neuronxcc compile every run)."""
    import hashlib
    import os
    import shutil
    import concourse.bass_utils as bu
    import concourse.bass2jax as b2j

    cache_dir = os.environ.get(
        "BASS_NEFF_CACHE_DIR",
        os.path.join(os.path.dirname(os.path.abspath(__file__)), ".neff_cache"))
    os.makedirs(cache_dir, exist_ok=True)
    orig = bu.compile_bir_kernel

    def cached(bir_json, tmpdir, neff_name="file.neff"):
        key = hashlib.sha256(bir_json).hexdigest()[:32]
        hit = os.path.join(cache_dir, key + ".neff")
        dst = os.path.join(tmpdir, neff_name)
        if os.path.exists(hit):
            shutil.copy(hit, dst)
            return dst
        neff = orig(bir_json, tmpdir, neff_name=neff_name)
        try:
            shutil.copy(neff, hit)
        except OSError:
            pass
        return neff

    bu.compile_bir_kernel = cached
    b2j.compile_bir_kernel = cached


_install_drain_split()
_install_profile_shim()
_install_neff_cache()


# ---------------------------------------------------------------------------
# Device program
# ---------------------------------------------------------------------------

def build(nc: bass.Bass, use_f32r=True, use_a2a=True):
    """Emit the per-core Tile program (SPMD: cores differ only in data)."""
    P = 128
    S, D = N, DIM
    ST = S // P          # 16 seq tiles
    DT = D // P          # 8 feat tiles
    NQ = S // 512        # 4 q chunks
    HD = HL * DH         # 256 local head cols
    QSL = S // LANES     # 512 output rows per core
    QT = QSL // P        # 4
    GROUPS = [list(range(N_CORES))]

    # fp32r matmul operands must be written pre-rounded by a compute op
    # (float32r-typed tiles); DMA output cannot feed an fp32r matmul.
    MMF = F32R if use_f32r else F32

    x_in = nc.dram_tensor("x", [S, D], F32, kind="ExternalInput").ap()
    wq_in = nc.dram_tensor("wq", [D, HD], F32, kind="ExternalInput").ap()
    wk_in = nc.dram_tensor("wk", [D, HD], F32, kind="ExternalInput").ap()
    wv_in = nc.dram_tensor("wv", [D, HD], F32, kind="ExternalInput").ap()
    gamma_in = nc.dram_tensor("gamma", [D], F32, kind="ExternalInput").ap()
    beta_in = nc.dram_tensor("beta", [D], F32, kind="ExternalInput").ap()
    if use_a2a:
        # doubled inner dim: row-block i holds lane (i%4)'s head rows, zeroed
        # for the other group's blocks (host builds this per core)
        wout_in = nc.dram_tensor("wout2", [2 * D, D], BF16,
                                 kind="ExternalInput").ap()
        out_dram = nc.dram_tensor("out", [QSL, D], F32,
                                  kind="ExternalOutput").ap()
        a2a_in = [nc.dram_tensor(f"a2a_in{h}", [N_CORES, DH, QSL], BF16).ap()
                  for h in range(HL)]
        a2a_out = [nc.dram_tensor(f"a2a_out{h}", [N_CORES, DH, QSL], BF16).ap()
                   for h in range(HL)]
        KTO = 2 * DT     # out-proj contraction tiles
    else:
        # no-collective fallback: emit the local partial product over the
        # core's 4 heads for ALL rows; host sums the 4 partials per batch.
        wout_in = nc.dram_tensor("woutp", [HD, D], BF16,
                                 kind="ExternalInput").ap()
        out_dram = nc.dram_tensor("out", [S, D], F32,
                                  kind="ExternalOutput").ap()
        KTO = HD // P    # 2

    with tile.TileContext(nc) as tc:
        with (
            tc.tile_pool(name="const", bufs=1) as const,
            tc.tile_pool(name="big", bufs=1) as big,
        ):
            # ---- small constants ----
            gamma_sb = const.tile([P, DT], F32)
            nc.sync.dma_start(out=gamma_sb,
                              in_=gamma_in.rearrange("(o p) -> p o", p=P))
            beta_sb = const.tile([P, DT], F32)
            nc.sync.dma_start(out=beta_sb,
                              in_=beta_in.rearrange("(o p) -> p o", p=P))
            eps_sb = const.tile([P, 1], F32)
            nc.vector.memset(eps_sb, EPS)
            ident = const.tile([P, P], F32)
            make_identity(nc, ident)

            # ---- activations that live through attention ----
            QT_sb = big.tile([P, HD // P, S], MMF)
            KT_sb = big.tile([P, HD // P, S], MMF)
            V_sb = big.tile([P, ST, HL, DH + 1], BF16)
            nc.vector.memset(V_sb[:, :, :, DH:DH + 1], 1.0)

            # ======== phases 1-3 (xnT scoped: freed before attention) ======
            with (
                tc.tile_pool(name="xnp", bufs=1) as xnp,
                tc.tile_pool(name="wstage", bufs=1) as wstage,
                tc.tile_pool(name="xp", bufs=3) as xp,
                tc.tile_pool(name="stats", bufs=4) as stats,
            ):
                xnT = xnp.tile([P, DT, S], MMF)

                def load_weight(name, src):
                    if use_f32r:
                        stage = wstage.tile([P, DT, HD], F32, tag="wstage",
                                            name=f"stage_{name}")
                        nc.sync.dma_start(
                            out=stage,
                            in_=src.rearrange("(o p) m -> p o m", p=P))
                        w = xnp.tile([P, DT, HD], F32R, tag=name, name=name)
                        nc.vector.tensor_copy(out=w, in_=stage)
                        return w
                    w = xnp.tile([P, DT, HD], F32, tag=name, name=name)
                    nc.sync.dma_start(
                        out=w, in_=src.rearrange("(o p) m -> p o m", p=P))
                    return w

                wq_sb = load_weight("wq", wq_in)
                wk_sb = load_weight("wk", wk_in)
                wv_sb = load_weight("wv", wv_in)

                # ---- phase 1+2: LayerNorm + transpose ----
                with tc.tile_pool(name="tp", bufs=4, space="PSUM") as tp:
                    for st in range(ST):
                        x_t = xp.tile([P, D], F32)
                        nc.sync.dma_start(out=x_t,
                                          in_=x_in[st * P:(st + 1) * P, :])
                        stt = stats.tile([P, 2, 6], F32)
                        nc.vector.bn_stats(out=stt[:, 0], in_=x_t[:, :D // 2])
                        nc.vector.bn_stats(out=stt[:, 1], in_=x_t[:, D // 2:])
                        mv = stats.tile([P, 2], F32)
                        nc.vector.bn_aggr(out=mv, in_=stt)
                        std = stats.tile([P, 1], F32)
                        nc.scalar.activation(
                            out=std, in_=mv[:, 1:2],
                            func=mybir.ActivationFunctionType.Sqrt, bias=eps_sb)
                        rstd = stats.tile([P, 1], F32)
                        nc.vector.reciprocal(out=rstd, in_=std)
                        nc.vector.tensor_scalar(
                            out=x_t, in0=x_t, scalar1=mv[:, 0:1], scalar2=rstd,
                            op0=mybir.AluOpType.subtract,
                            op1=mybir.AluOpType.mult)
                        for ft in range(DT):
                            pt_ps = tp.tile([P, P], F32, tag="tp")
                            nc.tensor.transpose(
                                pt_ps, x_t[:, ft * P:(ft + 1) * P], ident)
                            nc.vector.tensor_scalar(
                                out=xnT[:, ft, st * P:(st + 1) * P], in0=pt_ps,
                                scalar1=gamma_sb[:, ft:ft + 1],
                                scalar2=beta_sb[:, ft:ft + 1],
                                op0=mybir.AluOpType.mult,
                                op1=mybir.AluOpType.add)

                # ---- phase 3: projections ----
                with tc.tile_pool(name="proj", bufs=4, space="PSUM") as proj:
                    for w_sb, dst in ((wq_sb, QT_sb), (wk_sb, KT_sb)):
                        for pt in range(HD // P):
                            for nch in range(NQ):
                                ps = proj.tile([P, 512], F32, tag="proj")
                                for kt in range(DT):
                                    nc.tensor.matmul(
                                        ps, w_sb[:, kt, pt * P:(pt + 1) * P],
                                        xnT[:, kt, nch * 512:(nch + 1) * 512],
                                        start=(kt == 0), stop=(kt == DT - 1))
                                nc.vector.tensor_copy(
                                    out=dst[:, pt, nch * 512:(nch + 1) * 512],
                                    in_=ps)
                    for st in range(ST):
                        ps = proj.tile([P, HD], F32, tag="vproj")
                        for kt in range(DT):
                            nc.tensor.matmul(
                                ps, xnT[:, kt, st * P:(st + 1) * P],
                                wv_sb[:, kt, :],
                                start=(kt == 0), stop=(kt == DT - 1))
                        nc.vector.tensor_copy(
                            out=V_sb[:, st, :, 0:DH],
                            in_=ps.rearrange("p (h d) -> p h d", h=HL))

            # ======== phases 4-7 ========
            with (
                tc.tile_pool(name="late", bufs=1) as late,
                tc.tile_pool(name="expp", bufs=2) as expp,
                tc.tile_pool(name="obfp", bufs=2) as obfp,
                tc.tile_pool(name="bcast", bufs=2) as bcast,
                tc.tile_pool(name="outp", bufs=2) as outp,
            ):
                # out-proj weights: loaded while attention runs
                wout_sb = late.tile([P, KTO, D], BF16)
                nc.sync.dma_start(out=wout_sb,
                                  in_=wout_in.rearrange("(o p) m -> p o m", p=P))
                if use_a2a:
                    Ofull = late.tile([P, KTO, QSL], BF16)
                else:
                    obf_all = late.tile([P, KTO, S], BF16)

                with (
                    tc.tile_pool(name="spsum", bufs=2, space="PSUM") as spsum,
                    tc.tile_pool(name="opsum", bufs=NQ, space="PSUM") as opsum,
                ):
                    for h in range(HL):
                        kb = (h * DH) % P
                        kpt = (h * DH) // P
                        o_ps = [opsum.tile([DH + 1, 512], F32, tag="o",
                                           name=f"o_ps_{h}_{c}")
                                for c in range(NQ)]
                        for t in range(ST):
                            # q split in halves: QK^T of one half overlaps
                            # the ScalarE exp of the other (s_ps bufs=2)
                            for half in range(2):
                                s_ps = spsum.tile([P, S // 2], F32, tag="s",
                                                  name=f"s_ps_{h}_{t}_{half}")
                                for cc in range(NQ // 2):
                                    c = half * (NQ // 2) + cc
                                    nc.tensor.matmul(
                                        s_ps[:, cc * 512:(cc + 1) * 512],
                                        KT_sb[kb:kb + DH, kpt,
                                              t * P:(t + 1) * P],
                                        QT_sb[kb:kb + DH, kpt,
                                              c * 512:(c + 1) * 512],
                                        start=True, stop=True)
                                e_t = expp.tile([P, S // 2], BF16, tag="e",
                                                name=f"e_t_{h}_{t}_{half}")
                                nc.scalar.activation(
                                    out=e_t, in_=s_ps,
                                    func=mybir.ActivationFunctionType.Exp)
                                for cc in range(NQ // 2):
                                    c = half * (NQ // 2) + cc
                                    nc.tensor.matmul(
                                        o_ps[c], V_sb[:, t, h, :],
                                        e_t[:, cc * 512:(cc + 1) * 512],
                                        start=(t == 0), stop=(t == ST - 1))
                        # stage O_ext to SBUF at once: frees the 4 o_ps
                        # PSUM banks so the next head's attention overlaps
                        # the whole normalize + AllToAll chain below.
                        o_sb = bcast.tile([DH + 1, S], F32, tag="osum",
                                          name=f"o_sb_{h}")
                        for c in range(NQ):
                            nc.vector.tensor_copy(
                                out=o_sb[:, c * 512:(c + 1) * 512],
                                in_=o_ps[c])
                        # denominators on partition 64: reciprocal in place,
                        # broadcast to partitions 0..63 with doubling DMAs.
                        nc.vector.reciprocal(
                            out=o_sb[DH:DH + 1, :],
                            in_=o_sb[DH:DH + 1, :])
                        rec_b = bcast.tile([DH, S], F32, tag="rb")
                        nc.sync.dma_start(out=rec_b[0:1, :],
                                          in_=o_sb[DH:DH + 1, :])
                        k = 1
                        while k < DH:
                            nc.sync.dma_start(
                                out=rec_b[k:min(2 * k, DH), :],
                                in_=rec_b[0:min(k, DH - k), :])
                            k *= 2
                        if use_a2a:
                            obf_h = obfp.tile([DH, S], BF16, tag="obf")
                        else:
                            inner = h * DH
                            obf_h = obf_all[inner % P:inner % P + DH,
                                            inner // P, :]
                        nc.vector.tensor_mul(
                            out=obf_h, in0=o_sb[0:DH, :], in1=rec_b)
                        if use_a2a:
                            # lane shard j duplicated into both groups' slots
                            for half in range(2):
                                nc.sync.dma_start(
                                    out=a2a_in[h][half * LANES:
                                                  (half + 1) * LANES]
                                    .rearrange("j p q -> p j q"),
                                    in_=obf_h.rearrange(
                                        "p (j q) -> p j q", j=LANES))
                            nc.gpsimd.collective_compute(
                                "AllToAll", mybir.AluOpType.bypass,
                                replica_groups=GROUPS,
                                ins=[a2a_in[h][:]], outs=[a2a_out[h][:]])
                            for i in range(N_CORES):
                                inner = i * HD + h * DH
                                nc.gpsimd.dma_start(
                                    out=Ofull[inner % P:inner % P + DH,
                                              inner // P, :],
                                    in_=a2a_out[h][i])

                # ---- phase 7: output projection ----
                lhs = Ofull if use_a2a else obf_all
                n_qt = QT if use_a2a else ST
                with tc.tile_pool(name="oproj", bufs=4, space="PSUM") as oproj:
                    for qt in range(n_qt):
                        ot = outp.tile([P, D], F32, tag="ot")
                        for nch in range(D // 512):
                            ps = oproj.tile([P, 512], F32, tag="op")
                            for kt in range(KTO):
                                nc.tensor.matmul(
                                    ps, lhs[:, kt, qt * P:(qt + 1) * P],
                                    wout_sb[:, kt, nch * 512:(nch + 1) * 512],
                                    start=(kt == 0), stop=(kt == KTO - 1))
                            nc.vector.tensor_copy(
                                out=ot[:, nch * 512:(nch + 1) * 512], in_=ps)
                        nc.sync.dma_start(
                            out=out_dram[qt * P:(qt + 1) * P, :], in_=ot)

    return nc


# ---------------------------------------------------------------------------
# Host entry point
# ---------------------------------------------------------------------------

_CACHE = {}
USE_A2A = True
USE_F32R = True


def _get_program():
    key = (USE_A2A, USE_F32R)
    if key not in _CACHE:
        nc = bass.Bass("TRN2", target_bir_lowering=False, debug=False,
                       num_devices=N_CORES)
        build(nc, use_f32r=USE_F32R, use_a2a=USE_A2A)
        split_multi_waits(nc)
        _CACHE[key] = nc
    return _CACHE[key]


def _shard_inputs(x, gamma, beta, Wq, Wkv, Wout):
    import ml_dtypes
    x = np.asarray(x, dtype=np.float32)
    gamma = np.ascontiguousarray(np.asarray(gamma, dtype=np.float32))
    beta = np.ascontiguousarray(np.asarray(beta, dtype=np.float32))
    Wq = np.asarray(Wq, dtype=np.float32)
    Wkv = np.asarray(Wkv, dtype=np.float32)
    Wk, Wv = Wkv[:, :H * DH], Wkv[:, H * DH:]
    wout_bf = np.ascontiguousarray(np.asarray(Wout, np.float32)).astype(
        ml_dtypes.bfloat16)
    in_maps = []
    for core in range(N_CORES):
        b = core // LANES
        lane = core % LANES
        cs = slice(lane * HL * DH, (lane + 1) * HL * DH)
        m = {
            "x": np.ascontiguousarray(x[b]),
            "wq": np.ascontiguousarray(Wq[:, cs]),
            "wk": np.ascontiguousarray(Wk[:, cs]),
            "wv": np.ascontiguousarray(Wv[:, cs]),
            "gamma": gamma,
            "beta": beta,
        }
        if USE_A2A:
            wout2 = np.zeros((2 * DIM, DIM), dtype=ml_dtypes.bfloat16)
            gb = b * DIM
            wout2[gb:gb + DIM] = wout_bf
            m["wout2"] = wout2
        else:
            m["woutp"] = np.ascontiguousarray(wout_bf[cs.start:cs.stop])
        in_maps.append(m)
    return in_maps


def _unshard_output(results):
    out = np.empty((B, N, DIM), dtype=np.float32)
    if USE_A2A:
        qsl = N // LANES
        for core in range(N_CORES):
            b = core // LANES
            lane = core % LANES
            out[b, lane * qsl:(lane + 1) * qsl, :] = results[core]["out"]
    else:
        for b in range(B):
            acc = results[b * LANES]["out"].astype(np.float32).copy()
            for lane in range(1, LANES):
                acc += results[b * LANES + lane]["out"]
            out[b] = acc
    return out


def kernel(x, gamma, beta, Wq, Wkv, Wout, trace=False):
    from concourse.bass_utils import run_bass_kernel_spmd
    nc = _get_program()
    in_maps = _shard_inputs(x, gamma, beta, Wq, Wkv, Wout)
    res = run_bass_kernel_spmd(nc, in_maps, list(range(N_CORES)), trace=trace)
    out = _unshard_output(res.results)
    if trace:
        kernel.last_exec_time_ns = res.exec_time_ns
        kernel.last_result = res
    return out

